# revision 39
# baseline (speedup 1.0000x reference)
"""Multi-head attention (b=1, n=2048, d_model=1024, 16 heads x 64) on 8 TRN2
NeuronCores, head-parallel tensor parallelism: each core computes 2 heads end
to end (qkv projection for its heads, attention, and its slice of the output
projection); the 8 partial outputs (rank-128 slices of the out-proj
contraction) are summed on the host along with b_out.

Device kernel per core (bf16 compute, f32 accumulation in PSUM):
  A) x arrives pre-transposed + pre-cast on host: xT [1024, 2048] bf16,
     DMA'd in four 512-column chunks straight into SBUF (no PE transposes,
     no on-device casts).
  B) qT = (Wq/8)^T x^T (+ bq via per-partition tensor_scalar during the
     psum->SBUF copy); kT raw [128, 2048] (both heads' k-dims stacked);
     V natural (+ ones column for softmax row sums). K-bias is dropped
     (softmax shift invariance); V-bias is folded into b_out on the host
     (softmax rows sum to 1, so A += bv exactly).
  C) per 512-wide i-chunk: ONE merged matmul per key block j computes both
     heads' S^T into [128, 1024] (stationary = raw K^T block; moving = a
     zero-interleaved Q layout qz whose disjoint partition support keeps the
     heads separate) -> exp (ACT) -> P^T; A_aug = P^T^T V_aug accumulated
     over j in PSUM; rows normalized by reciprocal_approx_fast of the
     ones-column sum broadcast via an f32 rank-1 matmul.
  D) partial_out = A^T^T W_out_slice -> f16 out
"""

import os
import sys

sys.path.insert(0, "/opt/trn_rl_repo")

import numpy as np
import ml_dtypes

import concourse.bass as bass
import concourse.tile as tile
from concourse import bacc, mybir
from concourse.bass_utils import run_bass_kernel_spmd

F32 = mybir.dt.float32
F16 = mybir.dt.float16
BF16 = mybir.dt.bfloat16
F8 = mybir.dt.float8e4

N = 2048          # sequence length
D = 1024          # d_model
H_PER_CORE = 2    # heads per core
DH = 64           # head dim
C = H_PER_CORE * DH   # per-core qkv width = 128
N_CORES = 8
P = 128
N_TILES = N // P      # 16
D_TILES = D // P      # 8
I_CHUNK = 512         # query-chunk width for attention
N_ICHUNKS = N // I_CHUNK  # 4

_CACHE = {}


def build_graph():
    nc = bacc.Bacc()

    # host pre-arranges x^T and the weights into per-partition-contiguous
    # layouts so each DMA needs only 128 big descriptors (DGE descriptor
    # generation, not bandwidth, dominated the input latency otherwise).
    xt_ext = nc.declare_dram_parameter(
        "xt", [P, N_ICHUNKS * D_TILES * I_CHUNK], BF16, isOutput=False)
    wq_ext = nc.declare_dram_parameter("wq", [P, D_TILES * C], BF16, isOutput=False)
    wk_ext = nc.declare_dram_parameter("wk", [P, D_TILES * C], BF16, isOutput=False)
    wv_ext = nc.declare_dram_parameter("wv", [P, D_TILES * C], BF16, isOutput=False)
    wo_ext = nc.declare_dram_parameter("wo", [C, D], BF16, isOutput=False)
    bqt_ext = nc.declare_dram_parameter("bqt", [C, 1], F32, isOutput=False)
    out_ext = nc.declare_dram_parameter("out", [N, D], F16, isOutput=True)
    taps = {}
    if _CACHE.get("debug_taps"):
        for name, shape, dt in (
                ("tap_qT", [P, N], BF16), ("tap_kT", [P, N], BF16),
                ("tap_v", [P, N_TILES * 2 * (DH + 1)], BF16),
                ("tap_aT", [P, N], BF16), ("tap_pt0", [P, 2 * I_CHUNK], F8),
                ("tap_xT", [P, D_TILES * N], BF16)):
            taps[name] = nc.declare_dram_parameter(name, shape, dt,
                                                   isOutput=True)

    with tile.TileContext(nc) as tc:
        with (
            tc.tile_pool(name="persist", bufs=1) as persist,
            tc.tile_pool(name="pt", bufs=18) as ptpool,
            tc.tile_pool(name="small", bufs=6) as small,
            tc.tile_pool(name="outsb", bufs=4) as outsb,
            tc.tile_pool(name="ps_mm", bufs=2, space="PSUM") as ps_mm,
            tc.tile_pool(name="ps_s", bufs=2, space="PSUM") as ps_s,
            tc.tile_pool(name="ps_av", bufs=2, space="PSUM") as ps_av,
        ):
            # --- weights on the scalar HWDGE queue (ACT is idle at t=0 and
            # descriptor gen is now cheap); x^T chunk halves split across
            # the sync and gpsimd queues for 2x input bandwidth. Each DMA
            # is per-partition contiguous on both sides. ---
            wq_sb = persist.tile([P, D_TILES, C], BF16)
            wk_sb = persist.tile([P, D_TILES, C], BF16)
            wv_sb = persist.tile([P, D_TILES, C], BF16)
            wo_sb = persist.tile([C, D], BF16)
            bqt_sb = persist.tile([P, 1], F32)
            # x^T stored chunk-major: [partition, chunk, d-block, 512 cols]
            xT = persist.tile([P, N_ICHUNKS, D_TILES, I_CHUNK], BF16)
            nc.scalar.dma_start(
                wq_sb[:], wq_ext[:].rearrange("p (o c) -> p o c", c=C))
            nc.scalar.dma_start(
                wk_sb[:], wk_ext[:].rearrange("p (o c) -> p o c", c=C))
            nc.scalar.dma_start(bqt_sb[:], bqt_ext[:])
            nc.scalar.dma_start(
                wv_sb[:], wv_ext[:].rearrange("p (o c) -> p o c", c=C))
            nc.scalar.dma_start(wo_sb[:], wo_ext[:])
            csz = D_TILES * I_CHUNK  # elements per (partition, chunk)
            for ci in range(N_ICHUNKS):
                eng = (nc.sync, nc.gpsimd)[ci % 2]
                eng.dma_start(
                    xT[:, ci, :, :],
                    xt_ext[:, ci * csz:(ci + 1) * csz].rearrange(
                        "p (o n) -> p o n", n=I_CHUNK))

            kT = persist.tile([P, N], BF16)   # raw K^T, both heads stacked
            qT = persist.tile([P, N], BF16)   # Q^T + bias, both heads stacked
            # V in fp8e4, padded so each head's 65-wide block (64 V dims +
            # ones col at 64) starts at a 16B-aligned offset: h0 at 0,
            # h1 at 80. The j-pair stride (160B) is 16B-aligned as the
            # DoubleRow weight AP requires.
            VW = 160
            v_sb = persist.tile([P, N_TILES, VW], F8)
            nc.vector.memset(v_sb[:], 1.0)  # ones cols survive the copies
            aT = persist.tile([P, N], BF16)  # A^T, both heads stacked
            # P^T tiles in fp8e4, allocated in j-pairs so the AV matmul can
            # consume two key blocks per pass (fp8 DoubleRow: 256-deep
            # contraction, half the AV matmuls).
            st0 = {"pairs": [], "n": 0}
            st1 = {"pairs": [], "n": 0}

            def emit_score(cj, st):
                j = st["n"]
                sps = ps_s.tile([P, 2 * I_CHUNK], F32, tag="s_ps")
                jcols = slice(j * P, (j + 1) * P)
                ccols = slice(cj * I_CHUNK, (cj + 1) * I_CHUNK)
                # K=64 per head -> the two matmuls occupy disjoint PE
                # row-groups (0:64 / 64:128) and run concurrently.
                nc.tensor.matmul(sps[:, 0:I_CHUNK], kT[0:DH, jcols],
                                 qT[0:DH, ccols], start=True, stop=True)
                nc.tensor.matmul(sps[:, I_CHUNK:], kT[DH:P, jcols],
                                 qT[DH:P, ccols], start=True, stop=True)
                if j % 2 == 0:
                    st["pairs"].append(
                        ptpool.tile([P, 2, 2 * I_CHUNK], F8, tag="pt",
                                    name=f"ptp_{cj}_{j}"))
                nc.scalar.activation(
                    st["pairs"][-1][:, j % 2, :], sps[:],
                    mybir.ActivationFunctionType.Exp)
                st["n"] += 1

            def emit_av_pair(avps, prev_st, t):
                """One fp8 DoubleRow AV step: key blocks 2t, 2t+1 for both
                heads (contraction 256 = 128 partitions x j-pair)."""
                for h in range(H_PER_CORE):
                    nc.tensor.matmul(
                        avps[h][:],
                        v_sb[:, 2 * t:2 * t + 2, h * 80:h * 80 + DH + 1],
                        prev_st["pairs"][t][:, :, h * I_CHUNK:(h + 1) * I_CHUNK],
                        start=(t == 0), stop=(t == N_TILES // 2 - 1),
                        perf_mode=mybir.MatmulPerfMode.DoubleRow)

            # --- phase A+B: per 512-col chunk, project q/k (all 8 d-blocks)
            # and v (4 key tiles); then emit chunk-0 scores for those key
            # tiles so ACT starts exponentiating early.
            for ci in range(N_ICHUNKS):
                cols = slice(ci * I_CHUNK, (ci + 1) * I_CHUNK)
                qps = ps_s.tile([P, 2 * I_CHUNK], F32, tag="s_ps")
                for do in range(D_TILES):
                    nc.tensor.matmul(
                        qps[:, 0:I_CHUNK], wq_sb[:, do, :], xT[:, ci, do, :],
                        start=(do == 0), stop=(do == D_TILES - 1))
                nc.vector.tensor_scalar_add(
                    qT[:, cols], qps[:, 0:I_CHUNK], bqt_sb[:])
                kps = ps_s.tile([P, 2 * I_CHUNK], F32, tag="s_ps")
                for do in range(D_TILES):
                    nc.tensor.matmul(
                        kps[:, 0:I_CHUNK], wk_sb[:, do, :], xT[:, ci, do, :],
                        start=(do == 0), stop=(do == D_TILES - 1))
                nc.vector.tensor_copy(out=kT[:, cols], in_=kps[:, 0:I_CHUNK])
                for jt in range(4 * ci, 4 * ci + 4):
                    jo = (jt % 4) * P
                    vps = ps_mm.tile([P, 512], F32, tag="mm")
                    for do in range(D_TILES):
                        nc.tensor.matmul(
                            vps[:, 0:C], xT[:, ci, do, jo:jo + P],
                            wv_sb[:, do, :],
                            start=(do == 0), stop=(do == D_TILES - 1))
                    nc.vector.tensor_copy(out=v_sb[:, jt, 0:DH],
                                          in_=vps[:, 0:DH])
                    nc.vector.tensor_copy(out=v_sb[:, jt, 80:80 + DH],
                                          in_=vps[:, DH:C])
                # chunk-0 scores for this key group; from iteration 1 on,
                # also chunk-1 scores for the previous key group. This pulls
                # exp work into the projection phase where ACT would idle,
                # shrinking the ACT-paced attention phase.
                for _ in range(4):
                    emit_score(0, st0)
                if ci >= 1:
                    for _ in range(4):
                        emit_score(1, st1)

            # --- phases C+D: attention + out-proj per i-chunk ---
            def emit_qkchunk(ci, prev_st, st=None):
                """Emit S^T+exp of chunk ci interleaved with the DoubleRow
                AV pair-steps of chunk ci-1 (8 pairs spread over the odd
                score steps), so the PE fills exp-wait gaps with AV work."""
                if st is None:
                    st = {"pairs": [], "n": 0}
                avps = [ps_av.tile([DH + 1, I_CHUNK], F32, tag="av",
                                   name=f"av_{ci}_{h}")
                        for h in range(H_PER_CORE)]
                for j in range(N_TILES):
                    if st["n"] <= j:
                        emit_score(ci, st)
                    if j % 2 == 1:
                        emit_av_pair(avps, prev_st, j // 2)
                return st, avps

            def emit_norm_and_out(ci, avps):
                cols = slice(ci * I_CHUNK, (ci + 1) * I_CHUNK)
                for h in range(H_PER_CORE):
                    aps = avps[h]
                    # normalize: fast reciprocal of the denominator row,
                    # gpsimd partition-broadcast (SBUF->SBUF, idle engine),
                    # then one multiply straight to aT.
                    # copy the denominator row to SBUF partition 0 first: the
                    # custom-DVE reciprocal mishandles base partition > 0,
                    # and PSUM slice reads must be 32-aligned.
                    dsb = small.tile([1, I_CHUNK], F32, tag="dsb")
                    nc.vector.tensor_copy(out=dsb[:], in_=aps[DH:DH + 1, :])
                    rinv = small.tile([1, I_CHUNK], F32, tag="rinv")
                    nc.vector.reciprocal_approx_fast(rinv[:], dsb[:])
                    rbc_sb = small.tile([DH, I_CHUNK], F32, tag="rbc")
                    nc.gpsimd.partition_broadcast(rbc_sb[:], rinv[:])
                    nc.vector.tensor_tensor(
                        aT[h * DH:(h + 1) * DH, cols], aps[0:DH, :],
                        rbc_sb[:], mybir.AluOpType.mult)
                for ib in range(I_CHUNK // P):
                    iblk = ci * (I_CHUNK // P) + ib
                    osb = outsb.tile([P, D], F16, tag="osb")
                    for nn in range(2):
                        ops = ps_mm.tile([P, 512], F32, tag="mm")
                        nc.tensor.matmul(
                            ops[:], aT[:, iblk * P:(iblk + 1) * P],
                            wo_sb[:, nn * 512:(nn + 1) * 512],
                            start=True, stop=True)
                        nc.vector.tensor_copy(
                            out=osb[:, nn * 512:(nn + 1) * 512], in_=ops[:])
                    dma_eng = (nc.sync, nc.gpsimd)[iblk % 2]
                    dma_eng.dma_start(out_ext[iblk * P:(iblk + 1) * P, :],
                                      osb[:])

            def emit_qkchunk_last(prev_st):
                avps = [ps_av.tile([DH + 1, I_CHUNK], F32, tag="av",
                                   name=f"av_last_{h}")
                        for h in range(H_PER_CORE)]
                for t in range(N_TILES // 2):
                    emit_av_pair(avps, prev_st, t)
                return avps

            # chunk 0's scores (and most of chunk 1's) were fused into the
            # A/B loop.
            prev_st = st0
            for ci in range(1, N_ICHUNKS):
                st_next, avps = emit_qkchunk(
                    ci, prev_st, st1 if ci == 1 else None)
                emit_norm_and_out(ci - 1, avps)
                prev_st = st_next
            av_last = emit_qkchunk_last(prev_st)
            emit_norm_and_out(N_ICHUNKS - 1, av_last)
            if taps:
                nc.sync.dma_start(taps["tap_qT"][:], qT[:])
                nc.sync.dma_start(taps["tap_kT"][:], kT[:])
                nc.sync.dma_start(
                    taps["tap_v"][:],
                    v_sb[:].rearrange("p a b -> p (a b)"))
                nc.sync.dma_start(taps["tap_aT"][:], aT[:])
                nc.sync.dma_start(taps["tap_pt0"][:], st0["pairs"][0][:, 0, :])
                nc.sync.dma_start(
                    taps["tap_xT"][:],
                    xT[:].rearrange("p a b c -> p (a b c)"))
    nc.compile()
    return nc


def _to_pcontig(w):
    """[D, C] -> [P, D_TILES*C]: per-partition-contiguous weight layout
    (partition p holds rows p, p+128, ... of the original)."""
    return np.ascontiguousarray(
        w.reshape(D_TILES, P, C).transpose(1, 0, 2).reshape(P, D_TILES * C))


def _shard_inputs(x, W_qkv, b_qkv, W_out):
    x2d = np.asarray(x, dtype=np.float32).reshape(N, D)
    bf = ml_dtypes.bfloat16
    # x^T in chunk-major per-partition-contiguous layout:
    # xt[p, ci, o, n'] = x^T[o*128+p, ci*512+n']
    xt = np.ascontiguousarray(
        x2d.T.reshape(D_TILES, P, N_ICHUNKS, I_CHUNK)
        .transpose(1, 2, 0, 3).reshape(P, N_ICHUNKS * D_TILES * I_CHUNK)
        .astype(bf))
    Wr = np.asarray(W_qkv, dtype=np.float32).reshape(D, 3, 16, DH)
    br = np.asarray(b_qkv, dtype=np.float32).reshape(3, 16, DH)
    Wo = np.asarray(W_out, dtype=np.float32)
    scale = 1.0 / np.sqrt(DH)
    in_maps = []
    for c in range(N_CORES):
        hs = slice(2 * c, 2 * c + 2)
        in_maps.append({
            "xt": xt,
            "wq": _to_pcontig(
                Wr[:, 0, hs, :].reshape(D, C) * scale).astype(bf),
            "wk": _to_pcontig(Wr[:, 1, hs, :].reshape(D, C)).astype(bf),
            "wv": _to_pcontig(Wr[:, 2, hs, :].reshape(D, C)).astype(bf),
            "wo": np.ascontiguousarray(Wo[c * C:(c + 1) * C, :].astype(bf)),
            "bqt": np.ascontiguousarray(
                (br[0, hs, :].reshape(C, 1) * scale).astype(np.float32)),
        })
    return in_maps


def _install_profile_hook():
    """Recreate the antenv.axon_hooks NTFF profile hook missing from this
    image (same ctypes ABI the axon boot script uses), and neuter the
    artifact upload which needs credentials we don't have."""
    if _CACHE.get("hook"):
        return
    import contextlib
    import ctypes
    import types

    mod = types.ModuleType("antenv.axon_hooks")
    _state = {}
    mod.set_axon_ntff_profile_hook = lambda h: _state.__setitem__("h", h)
    mod.get_axon_ntff_profile_hook = lambda: _state.get("h")
    sys.modules["antenv.axon_hooks"] = mod

    so_path = os.environ.get("PJRT_LIBRARY_PATH", "/opt/axon/libaxon_pjrt.so")
    lib = ctypes.CDLL(so_path)
    lib.axon_start_nrt_profile.argtypes = [
        ctypes.POINTER(ctypes.c_int64), ctypes.c_size_t]
    lib.axon_start_nrt_profile.restype = ctypes.c_int64
    lib.axon_stop_nrt_profile.argtypes = [ctypes.c_char_p]
    lib.axon_stop_nrt_profile.restype = ctypes.c_int64

    @contextlib.contextmanager
    def _hook(output_dir, device_ids):
        import jax
        jax.devices()
        if device_ids:
            ids = (ctypes.c_int64 * len(device_ids))(*device_ids)
            rc = lib.axon_start_nrt_profile(ids, len(device_ids))
        else:
            rc = lib.axon_start_nrt_profile(None, 0)
        if rc != 0:
            raise RuntimeError(f"axon_start_nrt_profile rc={rc}")
        try:
            yield
        finally:
            n = lib.axon_stop_nrt_profile(str(output_dir).encode())
            print(f"profile: {n} file(s) written to {output_dir}")

    mod.set_axon_ntff_profile_hook(_hook)

    from concourse import bass_utils as bu
    bu.upload_artifacts = lambda tmpdir: str(tmpdir)
    _CACHE["hook"] = True


def run(inputs, trace=False):
    if trace:
        _install_profile_hook()
    if "nc" not in _CACHE:
        _CACHE["nc"] = build_graph()
    nc = _CACHE["nc"]
    in_maps = _shard_inputs(
        inputs["x"], inputs["W_qkv"], inputs["b_qkv"], inputs["W_out"])
    res = run_bass_kernel_spmd(nc, in_maps, list(range(N_CORES)), trace=trace)
    acc = np.zeros((N, D), dtype=np.float32)
    for m in res.results:
        acc += np.asarray(m["out"], dtype=np.float32)
    # host-side bias: b_out plus the V-bias pushed through the out
    # projection (softmax rows sum to 1, so A += bv exactly; K-bias
    # cancels in softmax entirely and Q-bias was applied on device).
    br = np.asarray(inputs["b_qkv"], dtype=np.float32).reshape(3, 16 * DH)
    Wo = np.asarray(inputs["W_out"], dtype=np.float32)
    acc += (np.asarray(inputs["b_out"], dtype=np.float32)
            + br[2] @ Wo)[None, :]
    return acc.reshape(1, N, D), res


def kernel(**inputs):
    out, _ = run(inputs, trace=False)
    return out


# revision 41
# speedup vs baseline: 1.1281x; 1.1281x over previous
"""Multi-head attention (b=1, n=2048, d_model=1024, 16 heads x 64) on 8 TRN2
NeuronCores, head-parallel tensor parallelism: each core computes 2 heads end
to end (qkv projection for its heads, attention, and its slice of the output
projection); the 8 partial outputs (rank-128 slices of the out-proj
contraction) are summed on the host along with b_out.

Device kernel per core (bf16 compute, f32 accumulation in PSUM):
  A) x arrives pre-transposed + pre-cast on host: xT [1024, 2048] bf16,
     DMA'd in four 512-column chunks straight into SBUF (no PE transposes,
     no on-device casts).
  B) qT = (Wq/8)^T x^T (+ bq via per-partition tensor_scalar during the
     psum->SBUF copy); kT raw [128, 2048] (both heads' k-dims stacked);
     V natural (+ ones column for softmax row sums). K-bias is dropped
     (softmax shift invariance); V-bias is folded into b_out on the host
     (softmax rows sum to 1, so A += bv exactly).
  C) per 512-wide i-chunk: ONE merged matmul per key block j computes both
     heads' S^T into [128, 1024] (stationary = raw K^T block; moving = a
     zero-interleaved Q layout qz whose disjoint partition support keeps the
     heads separate) -> exp (ACT) -> P^T; A_aug = P^T^T V_aug accumulated
     over j in PSUM; rows normalized by reciprocal_approx_fast of the
     ones-column sum broadcast via an f32 rank-1 matmul.
  D) partial_out = A^T^T W_out_slice -> f16 out
"""

import os
import sys

sys.path.insert(0, "/opt/trn_rl_repo")

import numpy as np
import ml_dtypes

import concourse.bass as bass
import concourse.tile as tile
from concourse import bacc, mybir
from concourse.bass_utils import run_bass_kernel_spmd

F32 = mybir.dt.float32
F16 = mybir.dt.float16
BF16 = mybir.dt.bfloat16
F8 = mybir.dt.float8e4

N = 2048          # sequence length
D = 1024          # d_model
H_PER_CORE = 2    # heads per core
DH = 64           # head dim
C = H_PER_CORE * DH   # per-core qkv width = 128
N_CORES = 8
P = 128
N_TILES = N // P      # 16
D_TILES = D // P      # 8
I_CHUNK = 512         # query-chunk width for attention
N_ICHUNKS = N // I_CHUNK  # 4

_CACHE = {}


def build_graph():
    nc = bacc.Bacc()

    # host pre-arranges x^T and the weights into per-partition-contiguous
    # layouts so each DMA needs only 128 big descriptors (DGE descriptor
    # generation, not bandwidth, dominated the input latency otherwise).
    xt_ext = nc.declare_dram_parameter(
        "xt", [P, N_ICHUNKS * D_TILES * I_CHUNK], BF16, isOutput=False)
    wq_ext = nc.declare_dram_parameter("wq", [P, D_TILES * C], BF16, isOutput=False)
    wk_ext = nc.declare_dram_parameter("wk", [P, D_TILES * C], BF16, isOutput=False)
    wv_ext = nc.declare_dram_parameter("wv", [P, D_TILES * C], BF16, isOutput=False)
    wo_ext = nc.declare_dram_parameter("wo", [C, D], BF16, isOutput=False)
    bqt_ext = nc.declare_dram_parameter("bqt", [C, 1], F32, isOutput=False)
    out_ext = nc.declare_dram_parameter("out", [N, D], F16, isOutput=True)
    taps = {}
    if _CACHE.get("debug_taps"):
        for name, shape, dt in (
                ("tap_qT", [P, N], BF16), ("tap_kT", [P, N], BF16),
                ("tap_v", [P, N_TILES * 2 * (DH + 1)], BF16),
                ("tap_aT", [P, N], BF16), ("tap_pt0", [P, 2 * I_CHUNK], BF16),
                ("tap_xT", [P, D_TILES * N], BF16)):
            taps[name] = nc.declare_dram_parameter(name, shape, dt,
                                                   isOutput=True)

    with tile.TileContext(nc) as tc:
        with (
            tc.tile_pool(name="persist", bufs=1) as persist,
            tc.tile_pool(name="pt", bufs=34) as ptpool,
            tc.tile_pool(name="small", bufs=6) as small,
            tc.tile_pool(name="outsb", bufs=4) as outsb,
            tc.tile_pool(name="ps_mm", bufs=2, space="PSUM") as ps_mm,
            tc.tile_pool(name="ps_s", bufs=2, space="PSUM") as ps_s,
            tc.tile_pool(name="ps_av", bufs=2, space="PSUM") as ps_av,
        ):
            # --- weights on the scalar HWDGE queue (ACT is idle at t=0 and
            # descriptor gen is now cheap); x^T chunk halves split across
            # the sync and gpsimd queues for 2x input bandwidth. Each DMA
            # is per-partition contiguous on both sides. ---
            wq_sb = persist.tile([P, D_TILES, C], BF16)
            wk_sb = persist.tile([P, D_TILES, C], BF16)
            wv_sb = persist.tile([P, D_TILES, C], BF16)
            wo_sb = persist.tile([C, D], BF16)
            bqt_sb = persist.tile([P, 1], F32)
            # x^T stored chunk-major: [partition, chunk, d-block, 512 cols]
            xT = persist.tile([P, N_ICHUNKS, D_TILES, I_CHUNK], BF16)
            nc.scalar.dma_start(
                wq_sb[:], wq_ext[:].rearrange("p (o c) -> p o c", c=C))
            nc.scalar.dma_start(
                wk_sb[:], wk_ext[:].rearrange("p (o c) -> p o c", c=C))
            nc.scalar.dma_start(bqt_sb[:], bqt_ext[:])
            nc.scalar.dma_start(
                wv_sb[:], wv_ext[:].rearrange("p (o c) -> p o c", c=C))
            nc.scalar.dma_start(wo_sb[:], wo_ext[:])
            csz = D_TILES * I_CHUNK  # elements per (partition, chunk)
            for ci in range(N_ICHUNKS):
                # both queues carry half of every chunk (split on the
                # d-block axis, keeping 4KB-contiguous runs), so chunks
                # land ~3us apart instead of ~6
                for eng, olo, ohi in ((nc.sync, 0, 4), (nc.gpsimd, 4, 8)):
                    lo = ci * csz + olo * I_CHUNK
                    eng.dma_start(
                        xT[:, ci, olo:ohi, :],
                        xt_ext[:, lo:lo + (ohi - olo) * I_CHUNK].rearrange(
                            "p (o n) -> p o n", n=I_CHUNK))

            kT = persist.tile([P, N], BF16)   # raw K^T, both heads stacked
            qT = persist.tile([P, N], BF16)   # Q^T + bias, both heads stacked
            v_sb = persist.tile([P, N_TILES, 2 * (DH + 1)], BF16)
            nc.vector.memset(v_sb[:], 1.0)  # ones cols survive the copies
            aT = persist.tile([P, N], BF16)  # A^T, both heads stacked
            st0 = {"tiles": [], "n": 0}
            st1 = {"tiles": [], "n": 0}

            def emit_score(cj, st):
                j = st["n"]
                sps = ps_s.tile([P, 2 * I_CHUNK], F32, tag="s_ps")
                jcols = slice(j * P, (j + 1) * P)
                ccols = slice(cj * I_CHUNK, (cj + 1) * I_CHUNK)
                # K=64 per head -> the two matmuls occupy disjoint PE
                # row-groups (0:64 / 64:128) and run concurrently.
                nc.tensor.matmul(sps[:, 0:I_CHUNK], kT[0:DH, jcols],
                                 qT[0:DH, ccols], start=True, stop=True)
                nc.tensor.matmul(sps[:, I_CHUNK:], kT[DH:P, jcols],
                                 qT[DH:P, ccols], start=True, stop=True)
                pt = ptpool.tile([P, 2 * I_CHUNK], BF16, tag="pt",
                                 name=f"pt_{cj}_{j}")
                st["tiles"].append(pt)
                nc.scalar.activation(
                    pt[:], sps[:], mybir.ActivationFunctionType.Exp)
                st["n"] += 1

            def emit_av(avps, prev_st, j):
                for h in range(H_PER_CORE):
                    nc.tensor.matmul(
                        avps[h][:],
                        v_sb[:, j, h * (DH + 1):(h + 1) * (DH + 1)],
                        prev_st["tiles"][j][:, h * I_CHUNK:(h + 1) * I_CHUNK],
                        start=(j == 0), stop=(j == N_TILES - 1))

            # --- phase A+B: per 512-col chunk, project q/k (all 8 d-blocks)
            # and v (4 key tiles); then emit chunk-0 scores for those key
            # tiles so ACT starts exponentiating early.
            for ci in range(N_ICHUNKS):
                cols = slice(ci * I_CHUNK, (ci + 1) * I_CHUNK)
                qps = ps_s.tile([P, 2 * I_CHUNK], F32, tag="s_ps")
                for do in range(D_TILES):
                    nc.tensor.matmul(
                        qps[:, 0:I_CHUNK], wq_sb[:, do, :], xT[:, ci, do, :],
                        start=(do == 0), stop=(do == D_TILES - 1))
                nc.vector.tensor_scalar_add(
                    qT[:, cols], qps[:, 0:I_CHUNK], bqt_sb[:])
                kps = ps_s.tile([P, 2 * I_CHUNK], F32, tag="s_ps")
                for do in range(D_TILES):
                    nc.tensor.matmul(
                        kps[:, 0:I_CHUNK], wk_sb[:, do, :], xT[:, ci, do, :],
                        start=(do == 0), stop=(do == D_TILES - 1))
                nc.vector.tensor_copy(out=kT[:, cols], in_=kps[:, 0:I_CHUNK])
                for jt in range(4 * ci, 4 * ci + 4):
                    jo = (jt % 4) * P
                    vps = ps_mm.tile([P, 512], F32, tag="mm")
                    for do in range(D_TILES):
                        nc.tensor.matmul(
                            vps[:, 0:C], xT[:, ci, do, jo:jo + P],
                            wv_sb[:, do, :],
                            start=(do == 0), stop=(do == D_TILES - 1))
                    nc.vector.tensor_copy(out=v_sb[:, jt, 0:DH],
                                          in_=vps[:, 0:DH])
                    nc.vector.tensor_copy(out=v_sb[:, jt, DH + 1:2 * DH + 1],
                                          in_=vps[:, DH:C])
                # chunk-0 scores for this key group; from iteration 1 on,
                # also chunk-1 scores for the previous key group. This pulls
                # exp work into the projection phase where ACT would idle,
                # shrinking the ACT-paced attention phase.
                for _ in range(4):
                    emit_score(0, st0)
                if ci >= 1:
                    for _ in range(4):
                        emit_score(1, st1)

            # --- phases C+D: attention + out-proj per i-chunk ---
            def emit_qkchunk(ci, prev_st, st=None, out_ci=None,
                             self_av=False):
                """Emit S^T+exp of chunk ci interleaved with the AV j-steps
                of chunk ci-1 (PE fills exp-wait gaps with AV work). May
                also interleave the previous chunk's out-proj blocks and,
                for the last chunk, its own AV steps at lag 2 (accumulated
                in ps_mm so ps_av keeps holding chunk ci-1)."""
                if st is None:
                    st = {"tiles": [], "n": 0}
                avps = [ps_av.tile([DH + 1, I_CHUNK], F32, tag="av",
                                   name=f"av_{ci}_{h}")
                        for h in range(H_PER_CORE)]
                avl = None
                if self_av:
                    avl = [ps_mm.tile([DH + 1, I_CHUNK], F32, tag="mm",
                                      name=f"avl_{h}")
                           for h in range(H_PER_CORE)]
                for j in range(N_TILES):
                    if st["n"] <= j:
                        emit_score(ci, st)
                    emit_av(avps, prev_st, j)
                    if out_ci is not None and j in (1, 3, 5, 7):
                        emit_out(out_ci, [(j - 1) // 2])
                    if avl is not None and j >= 2:
                        emit_av(avl, st, j - 2)
                return st, avps, avl

            def emit_norm(ci, avps):
                cols = slice(ci * I_CHUNK, (ci + 1) * I_CHUNK)
                for h in range(H_PER_CORE):
                    aps = avps[h]
                    # normalize: fast reciprocal of the denominator row,
                    # gpsimd partition-broadcast (SBUF->SBUF, idle engine),
                    # then one multiply straight to aT.
                    # copy the denominator row to SBUF partition 0 first: the
                    # custom-DVE reciprocal mishandles base partition > 0,
                    # and PSUM slice reads must be 32-aligned.
                    dsb = small.tile([1, I_CHUNK], F32, tag="dsb")
                    nc.vector.tensor_copy(out=dsb[:], in_=aps[DH:DH + 1, :])
                    rinv = small.tile([1, I_CHUNK], F32, tag="rinv")
                    nc.vector.reciprocal_approx_fast(rinv[:], dsb[:])
                    rbc_sb = small.tile([DH, I_CHUNK], F32, tag="rbc")
                    nc.gpsimd.partition_broadcast(rbc_sb[:], rinv[:])
                    nc.vector.tensor_tensor(
                        aT[h * DH:(h + 1) * DH, cols], aps[0:DH, :],
                        rbc_sb[:], mybir.AluOpType.mult)
            def emit_out(ci, ibs=None):
                for ib in (ibs if ibs is not None else range(I_CHUNK // P)):
                    iblk = ci * (I_CHUNK // P) + ib
                    osb = outsb.tile([P, D], F16, tag="osb")
                    for nn in range(2):
                        ops = ps_mm.tile([P, 512], F32, tag="mm")
                        nc.tensor.matmul(
                            ops[:], aT[:, iblk * P:(iblk + 1) * P],
                            wo_sb[:, nn * 512:(nn + 1) * 512],
                            start=True, stop=True)
                        nc.vector.tensor_copy(
                            out=osb[:, nn * 512:(nn + 1) * 512], in_=ops[:])
                    dma_eng = (nc.sync, nc.gpsimd)[iblk % 2]
                    dma_eng.dma_start(out_ext[iblk * P:(iblk + 1) * P, :],
                                      osb[:])

            # chunk 0's scores (and most of chunk 1's) were fused into the
            # A/B loop. Window 2 interleaves chunk 0's out-proj; window 3
            # interleaves chunk 3's own AV (lag 2) so only the norms and
            # out-projs remain after the last exp.
            st1b, avps0, _ = emit_qkchunk(1, st0, st1)
            emit_norm(0, avps0)
            st2, avps1, _ = emit_qkchunk(2, st1b, None, out_ci=0)
            emit_norm(1, avps1)
            emit_out(1)
            st3, avps2, avl = emit_qkchunk(3, st2, None, self_av=True)
            emit_norm(2, avps2)
            emit_av(avl, st3, N_TILES - 2)
            emit_av(avl, st3, N_TILES - 1)
            emit_norm(3, avl)
            emit_out(2)
            emit_out(3)
            if taps:
                nc.sync.dma_start(taps["tap_qT"][:], qT[:])
                nc.sync.dma_start(taps["tap_kT"][:], kT[:])
                nc.sync.dma_start(
                    taps["tap_v"][:],
                    v_sb[:].rearrange("p a b -> p (a b)"))
                nc.sync.dma_start(taps["tap_aT"][:], aT[:])
                nc.sync.dma_start(taps["tap_pt0"][:], st0["tiles"][0][:])
                nc.sync.dma_start(
                    taps["tap_xT"][:],
                    xT[:].rearrange("p a b c -> p (a b c)"))
    nc.compile()
    return nc


def _to_pcontig(w):
    """[D, C] -> [P, D_TILES*C]: per-partition-contiguous weight layout
    (partition p holds rows p, p+128, ... of the original)."""
    return np.ascontiguousarray(
        w.reshape(D_TILES, P, C).transpose(1, 0, 2).reshape(P, D_TILES * C))


def _shard_inputs(x, W_qkv, b_qkv, W_out):
    x2d = np.asarray(x, dtype=np.float32).reshape(N, D)
    bf = ml_dtypes.bfloat16
    # x^T in chunk-major per-partition-contiguous layout:
    # xt[p, ci, o, n'] = x^T[o*128+p, ci*512+n']
    xt = np.ascontiguousarray(
        x2d.T.reshape(D_TILES, P, N_ICHUNKS, I_CHUNK)
        .transpose(1, 2, 0, 3).reshape(P, N_ICHUNKS * D_TILES * I_CHUNK)
        .astype(bf))
    Wr = np.asarray(W_qkv, dtype=np.float32).reshape(D, 3, 16, DH)
    br = np.asarray(b_qkv, dtype=np.float32).reshape(3, 16, DH)
    Wo = np.asarray(W_out, dtype=np.float32)
    scale = 1.0 / np.sqrt(DH)
    in_maps = []
    for c in range(N_CORES):
        hs = slice(2 * c, 2 * c + 2)
        in_maps.append({
            "xt": xt,
            "wq": _to_pcontig(
                Wr[:, 0, hs, :].reshape(D, C) * scale).astype(bf),
            "wk": _to_pcontig(Wr[:, 1, hs, :].reshape(D, C)).astype(bf),
            "wv": _to_pcontig(Wr[:, 2, hs, :].reshape(D, C)).astype(bf),
            "wo": np.ascontiguousarray(Wo[c * C:(c + 1) * C, :].astype(bf)),
            "bqt": np.ascontiguousarray(
                (br[0, hs, :].reshape(C, 1) * scale).astype(np.float32)),
        })
    return in_maps


def _install_profile_hook():
    """Recreate the antenv.axon_hooks NTFF profile hook missing from this
    image (same ctypes ABI the axon boot script uses), and neuter the
    artifact upload which needs credentials we don't have."""
    if _CACHE.get("hook"):
        return
    import contextlib
    import ctypes
    import types

    mod = types.ModuleType("antenv.axon_hooks")
    _state = {}
    mod.set_axon_ntff_profile_hook = lambda h: _state.__setitem__("h", h)
    mod.get_axon_ntff_profile_hook = lambda: _state.get("h")
    sys.modules["antenv.axon_hooks"] = mod

    so_path = os.environ.get("PJRT_LIBRARY_PATH", "/opt/axon/libaxon_pjrt.so")
    lib = ctypes.CDLL(so_path)
    lib.axon_start_nrt_profile.argtypes = [
        ctypes.POINTER(ctypes.c_int64), ctypes.c_size_t]
    lib.axon_start_nrt_profile.restype = ctypes.c_int64
    lib.axon_stop_nrt_profile.argtypes = [ctypes.c_char_p]
    lib.axon_stop_nrt_profile.restype = ctypes.c_int64

    @contextlib.contextmanager
    def _hook(output_dir, device_ids):
        import jax
        jax.devices()
        if device_ids:
            ids = (ctypes.c_int64 * len(device_ids))(*device_ids)
            rc = lib.axon_start_nrt_profile(ids, len(device_ids))
        else:
            rc = lib.axon_start_nrt_profile(None, 0)
        if rc != 0:
            raise RuntimeError(f"axon_start_nrt_profile rc={rc}")
        try:
            yield
        finally:
            n = lib.axon_stop_nrt_profile(str(output_dir).encode())
            print(f"profile: {n} file(s) written to {output_dir}")

    mod.set_axon_ntff_profile_hook(_hook)

    from concourse import bass_utils as bu
    bu.upload_artifacts = lambda tmpdir: str(tmpdir)
    _CACHE["hook"] = True


def run(inputs, trace=False):
    if trace:
        _install_profile_hook()
    if "nc" not in _CACHE:
        _CACHE["nc"] = build_graph()
    nc = _CACHE["nc"]
    in_maps = _shard_inputs(
        inputs["x"], inputs["W_qkv"], inputs["b_qkv"], inputs["W_out"])
    res = run_bass_kernel_spmd(nc, in_maps, list(range(N_CORES)), trace=trace)
    acc = np.zeros((N, D), dtype=np.float32)
    for m in res.results:
        acc += np.asarray(m["out"], dtype=np.float32)
    # host-side bias: b_out plus the V-bias pushed through the out
    # projection (softmax rows sum to 1, so A += bv exactly; K-bias
    # cancels in softmax entirely and Q-bias was applied on device).
    br = np.asarray(inputs["b_qkv"], dtype=np.float32).reshape(3, 16 * DH)
    Wo = np.asarray(inputs["W_out"], dtype=np.float32)
    acc += (np.asarray(inputs["b_out"], dtype=np.float32)
            + br[2] @ Wo)[None, :]
    return acc.reshape(1, N, D), res


def kernel(**inputs):
    out, _ = run(inputs, trace=False)
    return out


# revision 43
# speedup vs baseline: 1.1543x; 1.0232x over previous
"""Multi-head attention (b=1, n=2048, d_model=1024, 16 heads x 64) on 8 TRN2
NeuronCores, head-parallel tensor parallelism: each core computes 2 heads end
to end (qkv projection for its heads, attention, and its slice of the output
projection); the 8 partial outputs (rank-128 slices of the out-proj
contraction) are summed on the host along with b_out.

Device kernel per core (bf16 compute, f32 accumulation in PSUM):
  A) x arrives pre-transposed + pre-cast on host: xT [1024, 2048] bf16,
     DMA'd in four 512-column chunks straight into SBUF (no PE transposes,
     no on-device casts).
  B) qT = (Wq/8)^T x^T (+ bq via per-partition tensor_scalar during the
     psum->SBUF copy); kT raw [128, 2048] (both heads' k-dims stacked);
     V natural (+ ones column for softmax row sums). K-bias is dropped
     (softmax shift invariance); V-bias is folded into b_out on the host
     (softmax rows sum to 1, so A += bv exactly).
  C) per 512-wide i-chunk: ONE merged matmul per key block j computes both
     heads' S^T into [128, 1024] (stationary = raw K^T block; moving = a
     zero-interleaved Q layout qz whose disjoint partition support keeps the
     heads separate) -> exp (ACT) -> P^T; A_aug = P^T^T V_aug accumulated
     over j in PSUM; rows normalized by reciprocal_approx_fast of the
     ones-column sum broadcast via an f32 rank-1 matmul.
  D) partial_out = A^T^T W_out_slice -> f16 out
"""

import os
import sys

sys.path.insert(0, "/opt/trn_rl_repo")

import numpy as np
import ml_dtypes

import concourse.bass as bass
import concourse.tile as tile
from concourse import bacc, mybir
from concourse.bass_utils import run_bass_kernel_spmd

F32 = mybir.dt.float32
F16 = mybir.dt.float16
BF16 = mybir.dt.bfloat16
F8 = mybir.dt.float8e4

N = 2048          # sequence length
D = 1024          # d_model
H_PER_CORE = 2    # heads per core
DH = 64           # head dim
C = H_PER_CORE * DH   # per-core qkv width = 128
N_CORES = 8
P = 128
N_TILES = N // P      # 16
D_TILES = D // P      # 8
I_CHUNK = 512         # query-chunk width for attention
N_ICHUNKS = N // I_CHUNK  # 4

_CACHE = {}


def build_graph():
    nc = bacc.Bacc()

    # host pre-arranges x^T and the weights into per-partition-contiguous
    # layouts so each DMA needs only 128 big descriptors (DGE descriptor
    # generation, not bandwidth, dominated the input latency otherwise).
    xt_ext = nc.declare_dram_parameter(
        "xt", [P, N_ICHUNKS * D_TILES * I_CHUNK], BF16, isOutput=False)
    wq_ext = nc.declare_dram_parameter("wq", [P, D_TILES * C], BF16, isOutput=False)
    wk_ext = nc.declare_dram_parameter("wk", [P, D_TILES * C], BF16, isOutput=False)
    wv_ext = nc.declare_dram_parameter("wv", [P, D_TILES * C], BF16, isOutput=False)
    wo_ext = nc.declare_dram_parameter("wo", [C, D], BF16, isOutput=False)
    bqt_ext = nc.declare_dram_parameter("bqt", [C, 1], F32, isOutput=False)
    out_ext = nc.declare_dram_parameter("out", [N, D], F16, isOutput=True)
    taps = {}
    if _CACHE.get("debug_taps"):
        for name, shape, dt in (
                ("tap_qT", [P, N], BF16), ("tap_kT", [P, N], BF16),
                ("tap_v", [P, N_TILES * 2 * (DH + 1)], BF16),
                ("tap_aT", [P, N], BF16), ("tap_pt0", [P, 2 * I_CHUNK], BF16),
                ("tap_xT", [P, D_TILES * N], BF16)):
            taps[name] = nc.declare_dram_parameter(name, shape, dt,
                                                   isOutput=True)

    with tile.TileContext(nc) as tc:
        with (
            tc.tile_pool(name="persist", bufs=1) as persist,
            tc.tile_pool(name="pt", bufs=34) as ptpool,
            tc.tile_pool(name="small", bufs=6) as small,
            tc.tile_pool(name="outsb", bufs=4) as outsb,
            tc.tile_pool(name="ps_mm", bufs=2, space="PSUM") as ps_mm,
            tc.tile_pool(name="ps_s", bufs=2, space="PSUM") as ps_s,
            tc.tile_pool(name="ps_av", bufs=2, space="PSUM") as ps_av,
        ):
            # --- weights on the scalar HWDGE queue (ACT is idle at t=0 and
            # descriptor gen is now cheap); x^T chunk halves split across
            # the sync and gpsimd queues for 2x input bandwidth. Each DMA
            # is per-partition contiguous on both sides. ---
            wq_sb = persist.tile([P, D_TILES, C], BF16)
            wk_sb = persist.tile([P, D_TILES, C], BF16)
            wv_sb = persist.tile([P, D_TILES, C], BF16)
            wo_sb = persist.tile([C, D], BF16)
            bqt_sb = persist.tile([P, 1], F32)
            # x^T stored chunk-major: [partition, chunk, d-block, 512 cols]
            xT = persist.tile([P, N_ICHUNKS, D_TILES, I_CHUNK], BF16)
            nc.scalar.dma_start(
                wq_sb[:], wq_ext[:].rearrange("p (o c) -> p o c", c=C))
            nc.scalar.dma_start(
                wk_sb[:], wk_ext[:].rearrange("p (o c) -> p o c", c=C))
            nc.scalar.dma_start(bqt_sb[:], bqt_ext[:])
            nc.scalar.dma_start(
                wv_sb[:], wv_ext[:].rearrange("p (o c) -> p o c", c=C))
            nc.scalar.dma_start(wo_sb[:], wo_ext[:])
            csz = D_TILES * I_CHUNK  # elements per (partition, chunk)
            for ci in range(N_ICHUNKS):
                eng = (nc.sync, nc.gpsimd)[ci % 2]
                eng.dma_start(
                    xT[:, ci, :, :],
                    xt_ext[:, ci * csz:(ci + 1) * csz].rearrange(
                        "p (o n) -> p o n", n=I_CHUNK))

            kT = persist.tile([P, N], BF16)   # raw K^T, both heads stacked
            qT = persist.tile([P, N], BF16)   # Q^T + bias, both heads stacked
            v_sb = persist.tile([P, N_TILES, 2 * (DH + 1)], BF16)
            nc.vector.memset(v_sb[:], 1.0)  # ones cols survive the copies
            aT = persist.tile([P, N], BF16)  # A^T, both heads stacked
            st0 = {"tiles": [], "n": 0}
            st1 = {"tiles": [], "n": 0}

            def emit_score(cj, st):
                j = st["n"]
                sps = ps_s.tile([P, 2 * I_CHUNK], F32, tag="s_ps")
                jcols = slice(j * P, (j + 1) * P)
                ccols = slice(cj * I_CHUNK, (cj + 1) * I_CHUNK)
                # K=64 per head -> the two matmuls occupy disjoint PE
                # row-groups (0:64 / 64:128) and run concurrently.
                nc.tensor.matmul(sps[:, 0:I_CHUNK], kT[0:DH, jcols],
                                 qT[0:DH, ccols], start=True, stop=True)
                nc.tensor.matmul(sps[:, I_CHUNK:], kT[DH:P, jcols],
                                 qT[DH:P, ccols], start=True, stop=True)
                pt = ptpool.tile([P, 2 * I_CHUNK], BF16, tag="pt",
                                 name=f"pt_{cj}_{j}")
                st["tiles"].append(pt)
                nc.scalar.activation(
                    pt[:], sps[:], mybir.ActivationFunctionType.Exp)
                st["n"] += 1

            def emit_av(avps, prev_st, j):
                for h in range(H_PER_CORE):
                    nc.tensor.matmul(
                        avps[h][:],
                        v_sb[:, j, h * (DH + 1):(h + 1) * (DH + 1)],
                        prev_st["tiles"][j][:, h * I_CHUNK:(h + 1) * I_CHUNK],
                        start=(j == 0), stop=(j == N_TILES - 1))

            # --- phase A+B: per 512-col chunk, project q/k (all 8 d-blocks)
            # and v (4 key tiles); then emit chunk-0 scores for those key
            # tiles so ACT starts exponentiating early.
            for ci in range(N_ICHUNKS):
                cols = slice(ci * I_CHUNK, (ci + 1) * I_CHUNK)
                qps = ps_s.tile([P, 2 * I_CHUNK], F32, tag="s_ps")
                for do in range(D_TILES):
                    nc.tensor.matmul(
                        qps[:, 0:I_CHUNK], wq_sb[:, do, :], xT[:, ci, do, :],
                        start=(do == 0), stop=(do == D_TILES - 1))
                nc.vector.tensor_scalar_add(
                    qT[:, cols], qps[:, 0:I_CHUNK], bqt_sb[:])
                kps = ps_s.tile([P, 2 * I_CHUNK], F32, tag="s_ps")
                for do in range(D_TILES):
                    nc.tensor.matmul(
                        kps[:, 0:I_CHUNK], wk_sb[:, do, :], xT[:, ci, do, :],
                        start=(do == 0), stop=(do == D_TILES - 1))
                nc.vector.tensor_copy(out=kT[:, cols], in_=kps[:, 0:I_CHUNK])
                for jt in range(4 * ci, 4 * ci + 4):
                    jo = (jt % 4) * P
                    vps = ps_mm.tile([P, 512], F32, tag="mm")
                    for do in range(D_TILES):
                        nc.tensor.matmul(
                            vps[:, 0:C], xT[:, ci, do, jo:jo + P],
                            wv_sb[:, do, :],
                            start=(do == 0), stop=(do == D_TILES - 1))
                    nc.vector.tensor_copy(out=v_sb[:, jt, 0:DH],
                                          in_=vps[:, 0:DH])
                    nc.vector.tensor_copy(out=v_sb[:, jt, DH + 1:2 * DH + 1],
                                          in_=vps[:, DH:C])
                # chunk-0 scores for this key group; from iteration 1 on,
                # also chunk-1 scores for the previous key group. This pulls
                # exp work into the projection phase where ACT would idle,
                # shrinking the ACT-paced attention phase.
                for _ in range(4):
                    emit_score(0, st0)
                if ci >= 1:
                    for _ in range(4):
                        emit_score(1, st1)

            # --- phases C+D: attention + out-proj per i-chunk ---
            def emit_qkchunk(ci, prev_st, st=None):
                """Emit S^T+exp of chunk ci interleaved with the AV j-steps
                of chunk ci-1, so the PE fills exp-wait gaps with AV work."""
                if st is None:
                    st = {"tiles": [], "n": 0}
                avps = [ps_av.tile([DH + 1, I_CHUNK], F32, tag="av",
                                   name=f"av_{ci}_{h}")
                        for h in range(H_PER_CORE)]
                for j in range(N_TILES):
                    if st["n"] <= j:
                        emit_score(ci, st)
                    emit_av(avps, prev_st, j)
                return st, avps

            def emit_norm_and_out(ci, avps):
                cols = slice(ci * I_CHUNK, (ci + 1) * I_CHUNK)
                for h in range(H_PER_CORE):
                    aps = avps[h]
                    # normalize: fast reciprocal of the denominator row,
                    # gpsimd partition-broadcast (SBUF->SBUF, idle engine),
                    # then one multiply straight to aT.
                    # copy the denominator row to SBUF partition 0 first: the
                    # custom-DVE reciprocal mishandles base partition > 0,
                    # and PSUM slice reads must be 32-aligned.
                    dsb = small.tile([1, I_CHUNK], F32, tag="dsb")
                    nc.vector.tensor_copy(out=dsb[:], in_=aps[DH:DH + 1, :])
                    rinv = small.tile([1, I_CHUNK], F32, tag="rinv")
                    nc.vector.reciprocal_approx_fast(rinv[:], dsb[:])
                    rbc_sb = small.tile([DH, I_CHUNK], F32, tag="rbc")
                    nc.gpsimd.partition_broadcast(rbc_sb[:], rinv[:])
                    nc.vector.tensor_tensor(
                        aT[h * DH:(h + 1) * DH, cols], aps[0:DH, :],
                        rbc_sb[:], mybir.AluOpType.mult)
                for ib in range(I_CHUNK // P):
                    iblk = ci * (I_CHUNK // P) + ib
                    osb = outsb.tile([P, D], F16, tag="osb")
                    for nn in range(2):
                        ops = ps_mm.tile([P, 512], F32, tag="mm")
                        nc.tensor.matmul(
                            ops[:], aT[:, iblk * P:(iblk + 1) * P],
                            wo_sb[:, nn * 512:(nn + 1) * 512],
                            start=True, stop=True)
                        nc.vector.tensor_copy(
                            out=osb[:, nn * 512:(nn + 1) * 512], in_=ops[:])
                    dma_eng = (nc.sync, nc.gpsimd)[iblk % 2]
                    dma_eng.dma_start(out_ext[iblk * P:(iblk + 1) * P, :],
                                      osb[:])

            def emit_qkchunk_last(prev_st):
                avps = [ps_av.tile([DH + 1, I_CHUNK], F32, tag="av",
                                   name=f"av_last_{h}")
                        for h in range(H_PER_CORE)]
                for j in range(N_TILES):
                    emit_av(avps, prev_st, j)
                return avps

            # chunk 0's scores (and most of chunk 1's) were fused into the
            # A/B loop.
            prev_st = st0
            for ci in range(1, N_ICHUNKS):
                st_next, avps = emit_qkchunk(
                    ci, prev_st, st1 if ci == 1 else None)
                emit_norm_and_out(ci - 1, avps)
                prev_st = st_next
            av_last = emit_qkchunk_last(prev_st)
            emit_norm_and_out(N_ICHUNKS - 1, av_last)
            if taps:
                nc.sync.dma_start(taps["tap_qT"][:], qT[:])
                nc.sync.dma_start(taps["tap_kT"][:], kT[:])
                nc.sync.dma_start(
                    taps["tap_v"][:],
                    v_sb[:].rearrange("p a b -> p (a b)"))
                nc.sync.dma_start(taps["tap_aT"][:], aT[:])
                nc.sync.dma_start(taps["tap_pt0"][:], st0["tiles"][0][:])
                nc.sync.dma_start(
                    taps["tap_xT"][:],
                    xT[:].rearrange("p a b c -> p (a b c)"))
    nc.compile()
    return nc


def _to_pcontig(w):
    """[D, C] -> [P, D_TILES*C]: per-partition-contiguous weight layout
    (partition p holds rows p, p+128, ... of the original)."""
    return np.ascontiguousarray(
        w.reshape(D_TILES, P, C).transpose(1, 0, 2).reshape(P, D_TILES * C))


def _shard_inputs(x, W_qkv, b_qkv, W_out):
    x2d = np.asarray(x, dtype=np.float32).reshape(N, D)
    bf = ml_dtypes.bfloat16
    # x^T in chunk-major per-partition-contiguous layout:
    # xt[p, ci, o, n'] = x^T[o*128+p, ci*512+n']
    xt = np.ascontiguousarray(
        x2d.T.reshape(D_TILES, P, N_ICHUNKS, I_CHUNK)
        .transpose(1, 2, 0, 3).reshape(P, N_ICHUNKS * D_TILES * I_CHUNK)
        .astype(bf))
    Wr = np.asarray(W_qkv, dtype=np.float32).reshape(D, 3, 16, DH)
    br = np.asarray(b_qkv, dtype=np.float32).reshape(3, 16, DH)
    Wo = np.asarray(W_out, dtype=np.float32)
    scale = 1.0 / np.sqrt(DH)
    in_maps = []
    for c in range(N_CORES):
        hs = slice(2 * c, 2 * c + 2)
        in_maps.append({
            "xt": xt,
            "wq": _to_pcontig(
                Wr[:, 0, hs, :].reshape(D, C) * scale).astype(bf),
            "wk": _to_pcontig(Wr[:, 1, hs, :].reshape(D, C)).astype(bf),
            "wv": _to_pcontig(Wr[:, 2, hs, :].reshape(D, C)).astype(bf),
            "wo": np.ascontiguousarray(Wo[c * C:(c + 1) * C, :].astype(bf)),
            "bqt": np.ascontiguousarray(
                (br[0, hs, :].reshape(C, 1) * scale).astype(np.float32)),
        })
    return in_maps


def _install_profile_hook():
    """Recreate the antenv.axon_hooks NTFF profile hook missing from this
    image (same ctypes ABI the axon boot script uses), and neuter the
    artifact upload which needs credentials we don't have."""
    if _CACHE.get("hook"):
        return
    import contextlib
    import ctypes
    import types

    mod = types.ModuleType("antenv.axon_hooks")
    _state = {}
    mod.set_axon_ntff_profile_hook = lambda h: _state.__setitem__("h", h)
    mod.get_axon_ntff_profile_hook = lambda: _state.get("h")
    sys.modules["antenv.axon_hooks"] = mod

    so_path = os.environ.get("PJRT_LIBRARY_PATH", "/opt/axon/libaxon_pjrt.so")
    lib = ctypes.CDLL(so_path)
    lib.axon_start_nrt_profile.argtypes = [
        ctypes.POINTER(ctypes.c_int64), ctypes.c_size_t]
    lib.axon_start_nrt_profile.restype = ctypes.c_int64
    lib.axon_stop_nrt_profile.argtypes = [ctypes.c_char_p]
    lib.axon_stop_nrt_profile.restype = ctypes.c_int64

    @contextlib.contextmanager
    def _hook(output_dir, device_ids):
        import jax
        jax.devices()
        if device_ids:
            ids = (ctypes.c_int64 * len(device_ids))(*device_ids)
            rc = lib.axon_start_nrt_profile(ids, len(device_ids))
        else:
            rc = lib.axon_start_nrt_profile(None, 0)
        if rc != 0:
            raise RuntimeError(f"axon_start_nrt_profile rc={rc}")
        try:
            yield
        finally:
            n = lib.axon_stop_nrt_profile(str(output_dir).encode())
            print(f"profile: {n} file(s) written to {output_dir}")

    mod.set_axon_ntff_profile_hook(_hook)

    from concourse import bass_utils as bu
    bu.upload_artifacts = lambda tmpdir: str(tmpdir)
    _CACHE["hook"] = True


def run(inputs, trace=False):
    if trace:
        _install_profile_hook()
    if "nc" not in _CACHE:
        _CACHE["nc"] = build_graph()
    nc = _CACHE["nc"]
    in_maps = _shard_inputs(
        inputs["x"], inputs["W_qkv"], inputs["b_qkv"], inputs["W_out"])
    res = run_bass_kernel_spmd(nc, in_maps, list(range(N_CORES)), trace=trace)
    acc = np.zeros((N, D), dtype=np.float32)
    for m in res.results:
        acc += np.asarray(m["out"], dtype=np.float32)
    # host-side bias: b_out plus the V-bias pushed through the out
    # projection (softmax rows sum to 1, so A += bv exactly; K-bias
    # cancels in softmax entirely and Q-bias was applied on device).
    br = np.asarray(inputs["b_qkv"], dtype=np.float32).reshape(3, 16 * DH)
    Wo = np.asarray(inputs["W_out"], dtype=np.float32)
    acc += (np.asarray(inputs["b_out"], dtype=np.float32)
            + br[2] @ Wo)[None, :]
    return acc.reshape(1, N, D), res


def kernel(**inputs):
    out, _ = run(inputs, trace=False)
    return out
